# revision 2
# baseline (speedup 1.0000x reference)
"""Attention graph convolution (GAT layer) on 8 TRN2 NeuronCores — v2.

Reference computation (all fp32):
    h   = input @ W                      # (N, 64)
    e   = leakyrelu(h@a1 + (h@a2).T)     # (N, N)
    att = softmax(where(adj>0, e, -inf)) # row softmax
    out = elu(att @ h)                   # (N, 64)

Sharding: rows of e/att (= output rows) are split across 8 cores,
no = 1536 rows each.  h (N x 64) is computed on every core (tiny).

v2 design (vs v1 baseline at 475 us):
  - adj is pre-transposed AND pre-cast to bf16 on the host per core:
    adjT[j, i] = adj[own_i, j].T.  This kills all 1152 per-core PE
    transposes, halves the dominant HBM read (604 MB -> 302 MB), and
    turns the mask multiply into an SBUF bf16 2x DVE op (was PSUM fp32
    1x).
  - input is pre-transposed to inputT [128, N] bf16 on the host and
    kept resident in SBUF, so each h chunk is a single PE matmul
    (no per-chunk DMA / PE transpose / PSUM copy).
  - everything on-chip is bf16 (fp32 PSUM accumulation): the big
    attention @ h matmul runs at 1 cycle/row instead of fp32's 4.
  - no max-subtraction softmax: |e| < ~30 so U = adjT * exp(lrelu(e))
    cannot overflow and equals the reference numerator up to the
    common exp(-max) factor.  P.T = h_ext.T @ U with h_ext = [h | 1];
    out = elu(P[:, :64] / P[:, 64]).
  - leakyrelu is split between ACT (Prelu, bias fused) and DVE
    (tensor_scalar + scalar_tensor_tensor) with fraction lrelu_act_frac
    on ACT to balance the two engines; exp and the mask multiply are
    batched over B=4 j-chunks per instruction to amortize per-op
    overheads.
"""

import numpy as np

N_TOTAL = 12288
K_IN = 128
F_OUT = 64
N_CORES = 8
ALPHA = 0.2


def build_program(
    nt: int,            # total nodes (j dim)
    no: int,            # nodes owned by this core (i dim)
    batch: int = 4,     # j-chunks per exp/mask instruction
    lrelu_act_frac: float = 0.35,  # j-chunk fraction with leakyrelu on ACT
    adjt_bufs: int = 3,
    ph1b_per_chunk: int = 2,
):
    from contextlib import ExitStack

    import concourse.bass as bass
    import concourse.mybir as mybir
    import concourse.tile as tile
    from concourse import bacc
    from concourse.alu_op_type import AluOpType
    from concourse.masks import make_identity

    f32 = mybir.dt.float32
    bf16 = mybir.dt.bfloat16
    AF = mybir.ActivationFunctionType

    P = 128
    F = F_OUT
    FE = F + 1                    # h columns + ones column
    K = K_IN
    assert nt % P == 0 and no % P == 0
    ncj = nt // P                 # j chunks (128 rows each)
    nic = no // P                 # i chunks (own rows)
    S = 512                       # i split for matmul free dim / psum banks
    ns = no // S
    assert no % S == 0
    B = batch
    NB = ncj // B
    assert ncj % B == 0
    n_act = int(round(lrelu_act_frac * ncj))
    NW = 4                        # inputT DMA split (column windows)
    assert ncj % NW == 0

    nc = bacc.Bacc("TRN2", target_bir_lowering=False, debug=False,
                   num_devices=1)

    inpT = nc.dram_tensor("inputT", [K, nt], bf16, kind="ExternalInput")
    inpT_own = nc.dram_tensor("inputT_own", [K, no], bf16,
                              kind="ExternalInput")
    adjT = nc.dram_tensor("adjT", [nt, no], bf16, kind="ExternalInput")
    w_d = nc.dram_tensor("W", [K, F], f32, kind="ExternalInput")
    a_d = nc.dram_tensor("a", [2 * F, 1], f32, kind="ExternalInput")
    out_d = nc.dram_tensor("out", [no, F], f32, kind="ExternalOutput")

    with tile.TileContext(nc) as tc, ExitStack() as ctx:
        consts = ctx.enter_context(tc.tile_pool(name="consts", bufs=1))

        identity = consts.tile([P, P], f32)
        make_identity(nc, identity)

        scr_ps = ctx.enter_context(
            tc.tile_pool(name="scr_ps", bufs=2, space="PSUM"))

        # ---- phase 0: Wa1 = W @ a1, Wa2 = W @ a2 (f32), cast bf16 ----
        wwa2_f = consts.tile([K, FE], f32)     # [W | Wa2]
        nc.sync.dma_start(wwa2_f[:, 0:F], w_d.ap())
        a_row = consts.tile([1, 2 * F], f32)   # a as a single-partition row
        nc.sync.dma_start(a_row[:], a_d.ap().rearrange("n o -> o n"))
        ito_own = consts.tile([K, no], bf16)   # inputT own window
        nc.sync.dma_start(ito_own[:], inpT_own.ap())

        ones_sb = consts.tile([P, P], f32)
        nc.vector.memset(ones_sb[:], 1.0)
        # replicate a across partitions via a K=1 matmul with a ones row
        a_rep = consts.tile([P, 2 * F], f32)
        a_rep_ps = scr_ps.tile([P, 2 * F], f32, tag="scr")
        nc.tensor.matmul(a_rep_ps[:], ones_sb[0:1, :], a_row[:],
                         start=True, stop=True)
        nc.vector.tensor_copy(a_rep[:], a_rep_ps[:])

        wa12_sb = consts.tile([K, 2], f32)
        wtmp = consts.tile([K, F], f32)
        nc.vector.tensor_tensor(wtmp[:], wwa2_f[:, 0:F], a_rep[:, 0:F],
                                AluOpType.mult)
        nc.vector.tensor_reduce(wa12_sb[:, 0:1], wtmp[:],
                                mybir.AxisListType.X, AluOpType.add)
        nc.vector.tensor_tensor(wtmp[:], wwa2_f[:, 0:F], a_rep[:, F:2 * F],
                                AluOpType.mult)
        nc.vector.tensor_reduce(wa12_sb[:, 1:2], wtmp[:],
                                mybir.AxisListType.X, AluOpType.add)
        nc.vector.tensor_copy(wwa2_f[:, F:FE], wa12_sb[:, 1:2])
        wwa2_bf = consts.tile([K, FE], bf16)   # [W | Wa2] bf16
        nc.vector.tensor_copy(wwa2_bf[:], wwa2_f[:])
        # Wa1 replicated to 128 cols, bf16
        wa1_rep_f = consts.tile([K, P], f32)
        nc.vector.tensor_scalar(wa1_rep_f[:], ones_sb[:], wa12_sb[:, 0:1],
                                None, AluOpType.mult)
        wa1_rep = consts.tile([K, P], bf16)
        nc.vector.tensor_copy(wa1_rep[:], wa1_rep_f[:])

        # ---- wh1_rep[p, i] = Wh1[own i] for all p ------------------------
        wh1_rep = consts.tile([P, no], bf16)
        for s in range(ns):
            w1p = scr_ps.tile([P, S], f32, tag="scr")
            nc.tensor.matmul(w1p[:], wa1_rep[:], ito_own[:, s * S:(s + 1) * S],
                             start=True, stop=True)
            nc.vector.tensor_copy(wh1_rep[:, s * S:(s + 1) * S], w1p[:])

        # ---- inputT resident in SBUF (windowed DMA) ----------------------
        ito_sb = consts.tile([K, nt], bf16)
        WCOL = nt // NW
        for w in range(NW):
            nc.sync.dma_start(ito_sb[:, w * WCOL:(w + 1) * WCOL],
                              inpT[:, w * WCOL:(w + 1) * WCOL])

        # ---- phase 1b: h_ext[:, jc, :] = [h | Wh2-col], wh2 --------------
        h_ext = consts.tile([P, ncj, FE], bf16)
        wh2_sb = consts.tile([P, ncj], f32)
        nc.vector.memset(h_ext[:, :, F], 1.0)

        def phase1b_chunk(jc):
            hw_ps = scr_ps.tile([P, FE], f32, tag="scr")
            nc.tensor.matmul(hw_ps[:], ito_sb[:, jc * P:(jc + 1) * P],
                             wwa2_bf[:], start=True, stop=True)
            nc.vector.tensor_copy(h_ext[:, jc, 0:F], hw_ps[:, 0:F])
            nc.vector.tensor_copy(wh2_sb[:, jc:jc + 1], hw_ps[:, F:FE])

        def act_path(jc):
            return (jc * 7919) % ncj < n_act

        # ---- phase 2: main loop over j batches ---------------------------
        pt_pool = ctx.enter_context(
            tc.tile_pool(name="pt_acc", bufs=1, space="PSUM"))
        pt_ps = pt_pool.tile([FE, no], f32)

        next_1b = 0
        with (
            tc.tile_pool(name="adjt", bufs=adjt_bufs) as adjt_pool,
            tc.tile_pool(name="epool", bufs=2) as e_pool,
            tc.tile_pool(name="upool", bufs=2) as u_pool,
        ):
            for b in range(NB):
                adjt = adjt_pool.tile([P, B, no], bf16, tag="adjt")
                nc.sync.dma_start(
                    adjt[:],
                    adjT[b * B * P:(b + 1) * B * P, :].rearrange(
                        "(q p) i -> p q i", p=P))
                e_sb = e_pool.tile([P, B, no], bf16, tag="e")
                for q in range(B):
                    jc = b * B + q
                    for _ in range(ph1b_per_chunk):
                        if next_1b < ncj:
                            phase1b_chunk(next_1b)
                            next_1b += 1
                    if act_path(jc):
                        nc.scalar.activation(e_sb[:, q, :], wh1_rep[:],
                                             AF.Prelu,
                                             bias=wh2_sb[:, jc:jc + 1],
                                             scale=1.0, alpha=ALPHA)
                    else:
                        # t = 0.2 * (Wh1 + Wh2) ; E = max(Wh1 + Wh2, t)
                        nc.vector.tensor_scalar(e_sb[:, q, :], wh1_rep[:],
                                                wh2_sb[:, jc:jc + 1], ALPHA,
                                                AluOpType.add, AluOpType.mult)
                        nc.vector.scalar_tensor_tensor(
                            e_sb[:, q, :], wh1_rep[:], wh2_sb[:, jc:jc + 1],
                            e_sb[:, q, :], AluOpType.add, AluOpType.max)
                nc.scalar.activation(e_sb[:], e_sb[:], AF.Exp)
                u_sb = u_pool.tile([P, B, no], bf16, tag="u")
                nc.vector.tensor_tensor(u_sb[:], e_sb[:], adjt[:],
                                        AluOpType.mult)
                for q in range(B):
                    jc = b * B + q
                    for s in range(ns):
                        nc.tensor.matmul(pt_ps[:, s * S:(s + 1) * S],
                                         h_ext[:, jc, :],
                                         u_sb[:, q, s * S:(s + 1) * S],
                                         start=(jc == 0),
                                         stop=(jc == ncj - 1))

        # ---- phase 3: out = elu(P[:, :64] / P[:, 64]) --------------------
        pt_sb = consts.tile([FE, no], f32)
        nc.vector.tensor_copy(pt_sb[:], pt_ps[:])
        with tc.tile_pool(name="fin_sb", bufs=4) as fin_sb:
            for ic in range(nic):
                ptp = scr_ps.tile([P, FE], f32, tag="scr")
                nc.tensor.transpose(ptp[:], pt_sb[:, ic * P:(ic + 1) * P],
                                    identity[0:FE, 0:FE])
                rec = fin_sb.tile([P, 1], f32, tag="rec")
                nc.vector.reciprocal(rec[:], ptp[:, F:FE])
                hp = fin_sb.tile([P, F], f32, tag="hp")
                nc.vector.tensor_scalar(hp[:], ptp[:, 0:F], rec[:], None,
                                        AluOpType.mult)
                # elu(x) = max(x,0) + exp(min(x,0)) - 1
                mn = fin_sb.tile([P, F], f32, tag="mn")
                nc.vector.tensor_scalar(mn[:], hp[:], 0.0, None, AluOpType.min)
                nc.scalar.activation(mn[:], mn[:], AF.Exp)
                nc.vector.tensor_scalar(hp[:], hp[:], 0.0, None, AluOpType.max)
                ob = fin_sb.tile([P, F], f32, tag="ob")
                nc.vector.scalar_tensor_tensor(
                    ob[:], mn[:], 1.0, hp[:],
                    AluOpType.subtract, AluOpType.add)
                nc.sync.dma_start(out_d[ic * P:(ic + 1) * P, :], ob[:])

    nc.compile()
    return nc


_CACHE = {}


def _get_program(nt, no, **kw):
    key = (nt, no, tuple(sorted(kw.items())))
    if key not in _CACHE:
        _CACHE[key] = build_program(nt, no, **kw)
    return _CACHE[key]


def prepare(inputs, **kw):
    """Build (program, per-core input maps) from full unsharded inputs."""
    import ml_dtypes

    bf16 = ml_dtypes.bfloat16
    input = np.ascontiguousarray(inputs["input"], dtype=np.float32)
    adj = inputs["adj"]
    W = np.ascontiguousarray(inputs["W"], dtype=np.float32)
    a = np.ascontiguousarray(inputs["a"], dtype=np.float32)

    nt = input.shape[0]
    no = nt // N_CORES
    nc = _get_program(nt, no, **kw)

    inputT = input.T.astype(bf16, order="C")
    in_maps = []
    for c in range(N_CORES):
        in_maps.append({
            "inputT": inputT,
            "inputT_own": np.ascontiguousarray(
                inputT[:, c * no:(c + 1) * no]),
            "adjT": adj[c * no:(c + 1) * no].T.astype(bf16, order="C"),
            "W": W,
            "a": a,
        })
    return nc, in_maps


def kernel(input, adj, W, a):
    from concourse.bass_utils import run_bass_kernel_spmd

    nc, in_maps = prepare({"input": input, "adj": adj, "W": W, "a": a})
    res = run_bass_kernel_spmd(nc, in_maps, list(range(N_CORES)))
    return np.concatenate([r["out"] for r in res.results], axis=0)


# revision 3
# speedup vs baseline: 1.0803x; 1.0803x over previous
"""Attention graph convolution (GAT layer) on 8 TRN2 NeuronCores — v2.1.

Reference computation (all fp32):
    h   = input @ W                      # (N, 64)
    e   = leakyrelu(h@a1 + (h@a2).T)     # (N, N)
    att = softmax(where(adj>0, e, -inf)) # row softmax
    out = elu(att @ h)                   # (N, 64)

Sharding: rows of e/att (= output rows) are split across 8 cores,
no = 1536 rows each.  h (N x 64) is computed on every core (tiny).

Design notes (v1 baseline 476 us -> v2 300 us -> this):
  - adj is pre-transposed AND pre-cast to bf16 on the host per core:
    adjT[j, i] = adj[own_i, j].T.  No PE transposes, half the HBM bytes,
    mask multiply is SBUF bf16 2x DVE (not PSUM fp32 1x).
  - input.T is host-prepared, bf16, SBUF-resident: each h chunk is one
    PE matmul.  Everything on-chip is bf16 (fp32 PSUM accumulation).
  - no max-subtraction softmax: |e| < ~30 so U = adjT * exp(lrelu(e))
    cannot overflow; P.T = h_ext.T @ U with h_ext = [h | 1];
    out = elu(P[:, :64] / P[:, 64]).
  - leakyrelu split between ACT (Prelu, bias fused) and DVE; the DVE
    path avoids scalar_tensor_tensor (1x-only uop, 1553 ns measured)
    in favor of TS + TS + TT (465+465+866, all 2x/4x modes).
  - mask multiply + the 3 accumulation matmuls are emitted per chunk so
    the PE never idles > ~2 us (HAM stays at 2.4 GHz; v2 measured cold
    matmuls at 440 ns vs 216 warm from 9 us idle gaps).
  - phase-1b h/Wh2 production is batched x4 through one PSUM tile with
    3D-AP copies (was 400 ns/chunk of DVE, now ~120).
  - optional gpsimd offload of the mask multiply for a fraction of
    chunks (gps_mask_frac) to shave the DVE bottleneck.
"""

import numpy as np

N_TOTAL = 12288
K_IN = 128
F_OUT = 64
N_CORES = 8
ALPHA = 0.2


def build_program(
    nt: int,            # total nodes (j dim)
    no: int,            # nodes owned by this core (i dim)
    batch: int = 4,     # j-chunks per exp instruction / phase1b group
    lrelu_act_frac: float = 0.42,  # j-chunk fraction with leakyrelu on ACT
    gps_mask_frac: float = 0.0,    # j-chunk fraction with mask on GPSIMD
    adjt_bufs: int = 3,
):
    from contextlib import ExitStack

    import concourse.bass as bass
    import concourse.mybir as mybir
    import concourse.tile as tile
    from concourse import bacc
    from concourse.alu_op_type import AluOpType
    from concourse.masks import make_identity

    f32 = mybir.dt.float32
    bf16 = mybir.dt.bfloat16
    AF = mybir.ActivationFunctionType

    P = 128
    F = F_OUT
    FE = F + 1                    # h columns + ones column
    K = K_IN
    assert nt % P == 0 and no % P == 0
    ncj = nt // P                 # j chunks (128 rows each)
    nic = no // P                 # i chunks (own rows)
    S = 512                       # i split for matmul free dim / psum banks
    ns = no // S
    assert no % S == 0
    B = batch
    NB = ncj // B
    assert ncj % B == 0
    n_act = int(round(lrelu_act_frac * ncj))
    n_gps = int(round(gps_mask_frac * ncj))
    NW = 4                        # inputT DMA split (column windows)

    nc = bacc.Bacc("TRN2", target_bir_lowering=False, debug=False,
                   num_devices=1)

    inpT = nc.dram_tensor("inputT", [K, nt], bf16, kind="ExternalInput")
    inpT_own = nc.dram_tensor("inputT_own", [K, no], bf16,
                              kind="ExternalInput")
    adjT = nc.dram_tensor("adjT", [nt, no], bf16, kind="ExternalInput")
    w_d = nc.dram_tensor("W", [K, F], f32, kind="ExternalInput")
    a_d = nc.dram_tensor("a", [2 * F, 1], f32, kind="ExternalInput")
    out_d = nc.dram_tensor("out", [no, F], f32, kind="ExternalOutput")

    with tile.TileContext(nc) as tc, ExitStack() as ctx:
        consts = ctx.enter_context(tc.tile_pool(name="consts", bufs=1))

        identity = consts.tile([P, P], f32)
        make_identity(nc, identity)

        scr_ps = ctx.enter_context(
            tc.tile_pool(name="scr_ps", bufs=2, space="PSUM"))
        p1b_ps = ctx.enter_context(
            tc.tile_pool(name="p1b_ps", bufs=2, space="PSUM"))

        # ---- phase 0: Wa1 = W @ a1, Wa2 = W @ a2 (f32), cast bf16 ----
        wwa2_f = consts.tile([K, FE], f32)     # [W | Wa2]
        nc.sync.dma_start(wwa2_f[:, 0:F], w_d.ap())
        a_row = consts.tile([1, 2 * F], f32)   # a as a single-partition row
        nc.sync.dma_start(a_row[:], a_d.ap().rearrange("n o -> o n"))
        ito_own = consts.tile([K, no], bf16)   # inputT own window
        nc.sync.dma_start(ito_own[:], inpT_own.ap())

        ones_sb = consts.tile([P, P], f32)
        nc.vector.memset(ones_sb[:], 1.0)
        # replicate a across partitions via a K=1 matmul with a ones row
        a_rep = consts.tile([P, 2 * F], f32)
        a_rep_ps = scr_ps.tile([P, 2 * F], f32, tag="scr")
        nc.tensor.matmul(a_rep_ps[:], ones_sb[0:1, :], a_row[:],
                         start=True, stop=True)
        nc.vector.tensor_copy(a_rep[:], a_rep_ps[:])

        wa12_sb = consts.tile([K, 2], f32)
        wtmp = consts.tile([K, F], f32)
        nc.vector.tensor_tensor(wtmp[:], wwa2_f[:, 0:F], a_rep[:, 0:F],
                                AluOpType.mult)
        nc.vector.tensor_reduce(wa12_sb[:, 0:1], wtmp[:],
                                mybir.AxisListType.X, AluOpType.add)
        nc.vector.tensor_tensor(wtmp[:], wwa2_f[:, 0:F], a_rep[:, F:2 * F],
                                AluOpType.mult)
        nc.vector.tensor_reduce(wa12_sb[:, 1:2], wtmp[:],
                                mybir.AxisListType.X, AluOpType.add)
        nc.vector.tensor_copy(wwa2_f[:, F:FE], wa12_sb[:, 1:2])
        wwa2_bf = consts.tile([K, FE], bf16)   # [W | Wa2] bf16
        nc.vector.tensor_copy(wwa2_bf[:], wwa2_f[:])
        # Wa1 replicated to 128 cols, bf16
        wa1_rep_f = consts.tile([K, P], f32)
        nc.vector.tensor_scalar(wa1_rep_f[:], ones_sb[:], wa12_sb[:, 0:1],
                                None, AluOpType.mult)
        wa1_rep = consts.tile([K, P], bf16)
        nc.vector.tensor_copy(wa1_rep[:], wa1_rep_f[:])

        # ---- wh1_rep[p, i] = Wh1[own i] for all p ------------------------
        wh1_rep = consts.tile([P, no], bf16)
        for s in range(ns):
            w1p = scr_ps.tile([P, S], f32, tag="scr")
            nc.tensor.matmul(w1p[:], wa1_rep[:], ito_own[:, s * S:(s + 1) * S],
                             start=True, stop=True)
            nc.vector.tensor_copy(wh1_rep[:, s * S:(s + 1) * S], w1p[:])

        # ---- inputT resident in SBUF (windowed DMA) ----------------------
        ito_sb = consts.tile([K, nt], bf16)
        WCOL = nt // NW
        for w in range(NW):
            nc.sync.dma_start(ito_sb[:, w * WCOL:(w + 1) * WCOL],
                              inpT[:, w * WCOL:(w + 1) * WCOL])

        # ---- phase 1b: h_ext[:, jc, :] = [h | Wh2-col], wh2 --------------
        h_ext = consts.tile([P, ncj, FE], bf16)
        wh2_sb = consts.tile([P, ncj], f32)
        nc.vector.memset(h_ext[:, :, F], 1.0)

        def phase1b_group(b):
            # B chunks' h/Wh2 through one PSUM tile, two 3D-AP copies
            hw_ps = p1b_ps.tile([P, B, FE], f32, tag="p1b")
            for q in range(B):
                jc = b * B + q
                nc.tensor.matmul(hw_ps[:, q, :], ito_sb[:, jc * P:(jc + 1) * P],
                                 wwa2_bf[:], start=True, stop=True)
            nc.vector.tensor_copy(h_ext[:, b * B:(b + 1) * B, 0:F],
                                  hw_ps[:, :, 0:F])
            nc.vector.tensor_copy(wh2_sb[:, b * B:(b + 1) * B],
                                  hw_ps[:, :, F])

        def act_path(jc):
            return (jc * 7919) % ncj < n_act

        def gps_path(jc):
            return (jc * 104729) % ncj < n_gps

        # ---- phase 2: main loop over j batches ---------------------------
        pt_pool = ctx.enter_context(
            tc.tile_pool(name="pt_acc", bufs=1, space="PSUM"))
        pt_ps = pt_pool.tile([FE, no], f32)

        with (
            tc.tile_pool(name="adjt", bufs=adjt_bufs) as adjt_pool,
            tc.tile_pool(name="epool", bufs=2) as e_pool,
            tc.tile_pool(name="tpool", bufs=2) as t_pool,
            tc.tile_pool(name="upool", bufs=2 * B) as u_pool,
        ):
            for b in range(NB):
                adjt = adjt_pool.tile([P, B, no], bf16, tag="adjt")
                nc.sync.dma_start(
                    adjt[:],
                    adjT[b * B * P:(b + 1) * B * P, :].rearrange(
                        "(q p) i -> p q i", p=P))
                phase1b_group(b)
                e_sb = e_pool.tile([P, B, no], bf16, tag="e")
                for q in range(B):
                    jc = b * B + q
                    if act_path(jc):
                        nc.scalar.activation(e_sb[:, q, :], wh1_rep[:],
                                             AF.Prelu,
                                             bias=wh2_sb[:, jc:jc + 1],
                                             scale=1.0, alpha=ALPHA)
                    else:
                        # x = Wh1 + Wh2 ; t = 0.2 x ; e = max(x, t)
                        t_sb = t_pool.tile([P, no], bf16, tag="t")
                        nc.vector.tensor_scalar(e_sb[:, q, :], wh1_rep[:],
                                                wh2_sb[:, jc:jc + 1], None,
                                                AluOpType.add)
                        nc.vector.tensor_scalar(t_sb[:], wh1_rep[:],
                                                wh2_sb[:, jc:jc + 1], ALPHA,
                                                AluOpType.add, AluOpType.mult)
                        nc.vector.tensor_tensor(e_sb[:, q, :], e_sb[:, q, :],
                                                t_sb[:], AluOpType.max)
                nc.scalar.activation(e_sb[:], e_sb[:], AF.Exp)
                for q in range(B):
                    jc = b * B + q
                    u_sb = u_pool.tile([P, no], bf16, tag="u")
                    eng = nc.gpsimd if gps_path(jc) else nc.vector
                    eng.tensor_tensor(u_sb[:], e_sb[:, q, :],
                                      adjt[:, q, :], AluOpType.mult)
                    for s in range(ns):
                        nc.tensor.matmul(pt_ps[:, s * S:(s + 1) * S],
                                         h_ext[:, jc, :],
                                         u_sb[:, s * S:(s + 1) * S],
                                         start=(jc == 0),
                                         stop=(jc == ncj - 1))

        # ---- phase 3: out = elu(P[:, :64] / P[:, 64]) --------------------
        pt_sb = consts.tile([FE, no], f32)
        nc.vector.tensor_copy(pt_sb[:], pt_ps[:])
        with tc.tile_pool(name="fin_sb", bufs=4) as fin_sb:
            for ic in range(nic):
                ptp = scr_ps.tile([P, FE], f32, tag="scr")
                nc.tensor.transpose(ptp[:], pt_sb[:, ic * P:(ic + 1) * P],
                                    identity[0:FE, 0:FE])
                rec = fin_sb.tile([P, 1], f32, tag="rec")
                nc.vector.reciprocal(rec[:], ptp[:, F:FE])
                hp = fin_sb.tile([P, F], f32, tag="hp")
                nc.vector.tensor_scalar(hp[:], ptp[:, 0:F], rec[:], None,
                                        AluOpType.mult)
                # elu(x) = max(x,0) + exp(min(x,0)) - 1
                mn = fin_sb.tile([P, F], f32, tag="mn")
                nc.vector.tensor_scalar(mn[:], hp[:], 0.0, None, AluOpType.min)
                nc.scalar.activation(mn[:], mn[:], AF.Exp)
                nc.vector.tensor_scalar(hp[:], hp[:], 0.0, None, AluOpType.max)
                ob = fin_sb.tile([P, F], f32, tag="ob")
                nc.vector.scalar_tensor_tensor(
                    ob[:], mn[:], 1.0, hp[:],
                    AluOpType.subtract, AluOpType.add)
                nc.sync.dma_start(out_d[ic * P:(ic + 1) * P, :], ob[:])

    nc.compile()
    return nc


_CACHE = {}


def _get_program(nt, no, **kw):
    key = (nt, no, tuple(sorted(kw.items())))
    if key not in _CACHE:
        _CACHE[key] = build_program(nt, no, **kw)
    return _CACHE[key]


def prepare(inputs, **kw):
    """Build (program, per-core input maps) from full unsharded inputs."""
    import ml_dtypes

    bf16 = ml_dtypes.bfloat16
    input = np.ascontiguousarray(inputs["input"], dtype=np.float32)
    adj = inputs["adj"]
    W = np.ascontiguousarray(inputs["W"], dtype=np.float32)
    a = np.ascontiguousarray(inputs["a"], dtype=np.float32)

    nt = input.shape[0]
    no = nt // N_CORES
    nc = _get_program(nt, no, **kw)

    inputT = input.T.astype(bf16, order="C")
    in_maps = []
    for c in range(N_CORES):
        in_maps.append({
            "inputT": inputT,
            "inputT_own": np.ascontiguousarray(
                inputT[:, c * no:(c + 1) * no]),
            "adjT": adj[c * no:(c + 1) * no].T.astype(bf16, order="C"),
            "W": W,
            "a": a,
        })
    return nc, in_maps


def kernel(input, adj, W, a):
    from concourse.bass_utils import run_bass_kernel_spmd

    nc, in_maps = prepare({"input": input, "adj": adj, "W": W, "a": a})
    res = run_bass_kernel_spmd(nc, in_maps, list(range(N_CORES)))
    return np.concatenate([r["out"] for r in res.results], axis=0)


# revision 4
# speedup vs baseline: 1.2905x; 1.1946x over previous
"""Attention graph convolution (GAT layer) on 8 TRN2 NeuronCores — v3.

Reference computation (all fp32):
    h   = input @ W                      # (N, 64)
    e   = leakyrelu(h@a1 + (h@a2).T)     # (N, N)
    att = softmax(where(adj>0, e, -inf)) # row softmax
    out = elu(att @ h)                   # (N, 64)

Sharding: rows of e/att (= output rows) are split across 8 cores,
no = 1536 rows each.  h (N x 64) is computed on every core (tiny).

Design (v1 476 us -> v2 300 -> v2.1 251 -> this):
  - the adjacency mask is host-baked as an ADDITIVE pre-activation
    offset M[j,i] = 0 (edge) / -150 (no edge), transposed to [j, i] and
    cast bf16.  Masking before the leakyrelu is exact enough:
    exp(lrelu(x-150)) <= e^-24, which is < 1e-10 of any row's softmax
    denominator.  This removes the post-exp mask multiply entirely.
  - a runtime-registered custom DVE op (dve_lrelu_op) fuses the whole
    pre-activation for a chunk into ONE Vector instruction:
        e = max(x, 0.2x),  x = Wh1_i + Wh2_j + M[j,i]
    (replaces tensor_scalar+tensor_scalar+tensor_tensor+mask multiply).
  - a lrelu_act_frac fraction of chunks instead run: tensor_tensor add
    (x+M) then ACT Prelu with the Wh2 bias folded in — balancing DVE
    vs ACT, whose irreducible job is the exp.
  - input.T is host-prepared bf16 and SBUF-resident; each h chunk is a
    single PE matmul; h/Wh2 copies are batched x4 through one PSUM
    tile with 3D-AP copies.
  - everything on-chip is bf16 (fp32 PSUM accumulation): the
    accumulation matmul streams at 1 cycle/row (fp32 is 4).
  - exp runs in x2-chunk sub-batches so the PE's accumulation matmuls
    arrive every ~2 us and HAM keeps the PE at 2.4 GHz.
  - no max-subtraction softmax: |e| < ~30 so U = exp(lrelu(e+M))
    cannot overflow; P.T = h_ext.T @ U with h_ext = [h | 1];
    out = elu(P[:, :64] / P[:, 64]).
"""

import numpy as np

N_TOTAL = 12288
K_IN = 128
F_OUT = 64
N_CORES = 8
ALPHA = 0.2
MASK_NEG = -150.0


def build_program(
    nt: int,            # total nodes (j dim)
    no: int,            # nodes owned by this core (i dim)
    batch: int = 4,     # j-chunks per adjacency DMA / phase1b group
    exp_sub: int = 2,   # j-chunks per exp instruction
    lrelu_act_frac: float = 0.27,  # j-chunk fraction with leakyrelu on ACT
    adjt_bufs: int = 3,
):
    from contextlib import ExitStack

    import concourse.bass as bass
    import concourse.mybir as mybir
    import concourse.tile as tile
    from concourse import bacc
    from concourse.alu_op_type import AluOpType
    from concourse.masks import make_identity

    import dve_lrelu_op

    LRELU_OP = dve_lrelu_op.get_op()

    f32 = mybir.dt.float32
    bf16 = mybir.dt.bfloat16
    AF = mybir.ActivationFunctionType

    P = 128
    F = F_OUT
    FE = F + 1                    # h columns + ones column
    K = K_IN
    assert nt % P == 0 and no % P == 0
    ncj = nt // P                 # j chunks (128 rows each)
    nic = no // P                 # i chunks (own rows)
    S = 512                       # i split for matmul free dim / psum banks
    ns = no // S
    assert no % S == 0
    B = batch
    NB = ncj // B
    assert ncj % B == 0 and B % exp_sub == 0
    n_act = int(round(lrelu_act_frac * ncj))
    NW = 4                        # inputT DMA split (column windows)

    nc = bacc.Bacc("TRN2", target_bir_lowering=False, debug=False,
                   num_devices=1)

    inpT = nc.dram_tensor("inputT", [K, nt], bf16, kind="ExternalInput")
    inpT_own = nc.dram_tensor("inputT_own", [K, no], bf16,
                              kind="ExternalInput")
    # additive mask, transposed: maskT[j, i] = 0 if adj[i, j] else -150
    maskT = nc.dram_tensor("maskT", [nt, no], bf16, kind="ExternalInput")
    w_d = nc.dram_tensor("W", [K, F], f32, kind="ExternalInput")
    a_d = nc.dram_tensor("a", [2 * F, 1], f32, kind="ExternalInput")
    out_d = nc.dram_tensor("out", [no, F], f32, kind="ExternalOutput")

    with tile.TileContext(nc) as tc, ExitStack() as ctx:
        consts = ctx.enter_context(tc.tile_pool(name="consts", bufs=1))

        identity = consts.tile([P, P], f32)
        make_identity(nc, identity)

        scr_ps = ctx.enter_context(
            tc.tile_pool(name="scr_ps", bufs=2, space="PSUM"))
        p1b_ps = ctx.enter_context(
            tc.tile_pool(name="p1b_ps", bufs=2, space="PSUM"))

        # ---- phase 0: Wa1 = W @ a1, Wa2 = W @ a2 (f32), cast bf16 ----
        wwa2_f = consts.tile([K, FE], f32)     # [W | Wa2]
        nc.sync.dma_start(wwa2_f[:, 0:F], w_d.ap())
        a_row = consts.tile([1, 2 * F], f32)   # a as a single-partition row
        nc.sync.dma_start(a_row[:], a_d.ap().rearrange("n o -> o n"))
        ito_own = consts.tile([K, no], bf16)   # inputT own window
        nc.sync.dma_start(ito_own[:], inpT_own.ap())

        ones_sb = consts.tile([P, P], f32)
        nc.vector.memset(ones_sb[:], 1.0)
        # replicate a across partitions via a K=1 matmul with a ones row
        a_rep = consts.tile([P, 2 * F], f32)
        a_rep_ps = scr_ps.tile([P, 2 * F], f32, tag="scr")
        nc.tensor.matmul(a_rep_ps[:], ones_sb[0:1, :], a_row[:],
                         start=True, stop=True)
        nc.vector.tensor_copy(a_rep[:], a_rep_ps[:])

        wa12_sb = consts.tile([K, 2], f32)
        wtmp = consts.tile([K, F], f32)
        nc.vector.tensor_tensor(wtmp[:], wwa2_f[:, 0:F], a_rep[:, 0:F],
                                AluOpType.mult)
        nc.vector.tensor_reduce(wa12_sb[:, 0:1], wtmp[:],
                                mybir.AxisListType.X, AluOpType.add)
        nc.vector.tensor_tensor(wtmp[:], wwa2_f[:, 0:F], a_rep[:, F:2 * F],
                                AluOpType.mult)
        nc.vector.tensor_reduce(wa12_sb[:, 1:2], wtmp[:],
                                mybir.AxisListType.X, AluOpType.add)
        nc.vector.tensor_copy(wwa2_f[:, F:FE], wa12_sb[:, 1:2])
        wwa2_bf = consts.tile([K, FE], bf16)   # [W | Wa2] bf16
        nc.vector.tensor_copy(wwa2_bf[:], wwa2_f[:])
        # Wa1 replicated to 128 cols, bf16
        wa1_rep_f = consts.tile([K, P], f32)
        nc.vector.tensor_scalar(wa1_rep_f[:], ones_sb[:], wa12_sb[:, 0:1],
                                None, AluOpType.mult)
        wa1_rep = consts.tile([K, P], bf16)
        nc.vector.tensor_copy(wa1_rep[:], wa1_rep_f[:])

        # ---- wh1_rep[p, i] = Wh1[own i] for all p ------------------------
        wh1_rep = consts.tile([P, no], bf16)
        for s in range(ns):
            w1p = scr_ps.tile([P, S], f32, tag="scr")
            nc.tensor.matmul(w1p[:], wa1_rep[:], ito_own[:, s * S:(s + 1) * S],
                             start=True, stop=True)
            nc.vector.tensor_copy(wh1_rep[:, s * S:(s + 1) * S], w1p[:])

        # ---- inputT resident in SBUF (windowed DMA) ----------------------
        ito_sb = consts.tile([K, nt], bf16)
        WCOL = nt // NW
        for w in range(NW):
            nc.sync.dma_start(ito_sb[:, w * WCOL:(w + 1) * WCOL],
                              inpT[:, w * WCOL:(w + 1) * WCOL])

        # ---- phase 1b: h_ext[:, jc, :] = [h | 1], wh2 --------------------
        h_ext = consts.tile([P, ncj, FE], bf16)
        wh2_sb = consts.tile([P, ncj], f32)
        nc.vector.memset(h_ext[:, :, F], 1.0)

        def phase1b_group(b):
            # B chunks' h/Wh2 through one PSUM tile, two 3D-AP copies
            hw_ps = p1b_ps.tile([P, B, FE], f32, tag="p1b")
            for q in range(B):
                jc = b * B + q
                nc.tensor.matmul(hw_ps[:, q, :], ito_sb[:, jc * P:(jc + 1) * P],
                                 wwa2_bf[:], start=True, stop=True)
            nc.vector.tensor_copy(h_ext[:, b * B:(b + 1) * B, 0:F],
                                  hw_ps[:, :, 0:F])
            nc.vector.tensor_copy(wh2_sb[:, b * B:(b + 1) * B],
                                  hw_ps[:, :, F])

        def act_path(jc):
            return (jc * 7919) % ncj < n_act

        # ---- phase 2: main loop over j batches ---------------------------
        pt_pool = ctx.enter_context(
            tc.tile_pool(name="pt_acc", bufs=1, space="PSUM"))
        pt_ps = pt_pool.tile([FE, no], f32)

        with (
            tc.tile_pool(name="adjt", bufs=adjt_bufs) as adjt_pool,
            tc.tile_pool(name="epool", bufs=2) as e_pool,
        ):
            for b in range(NB):
                adjt = adjt_pool.tile([P, B, no], bf16, tag="adjt")
                nc.sync.dma_start(
                    adjt[:],
                    maskT[b * B * P:(b + 1) * B * P, :].rearrange(
                        "(q p) i -> p q i", p=P))
                phase1b_group(b)
                e_sb = e_pool.tile([P, B, no], bf16, tag="e")
                for qs in range(B // exp_sub):
                    for q in range(qs * exp_sub, (qs + 1) * exp_sub):
                        jc = b * B + q
                        if act_path(jc):
                            # x+M on DVE, then lrelu with Wh2 bias on ACT
                            nc.vector.tensor_tensor(
                                e_sb[:, q, :], wh1_rep[:], adjt[:, q, :],
                                AluOpType.add)
                            nc.scalar.activation(
                                e_sb[:, q, :], e_sb[:, q, :], AF.Prelu,
                                bias=wh2_sb[:, jc:jc + 1],
                                scale=1.0, alpha=ALPHA)
                        else:
                            # one fused DVE op: max(x, 0.2x),
                            # x = Wh1 + Wh2 + M
                            nc.vector._custom_dve(
                                LRELU_OP, out=e_sb[:, q, :], in0=wh1_rep[:],
                                in1=adjt[:, q, :],
                                s0=wh2_sb[:, jc:jc + 1], s1=ALPHA)
                    nc.scalar.activation(
                        e_sb[:, qs * exp_sub:(qs + 1) * exp_sub, :],
                        e_sb[:, qs * exp_sub:(qs + 1) * exp_sub, :], AF.Exp)
                    for q in range(qs * exp_sub, (qs + 1) * exp_sub):
                        jc = b * B + q
                        for s in range(ns):
                            nc.tensor.matmul(pt_ps[:, s * S:(s + 1) * S],
                                             h_ext[:, jc, :],
                                             e_sb[:, q, s * S:(s + 1) * S],
                                             start=(jc == 0),
                                             stop=(jc == ncj - 1))

        # ---- phase 3: out = elu(P[:, :64] / P[:, 64]) --------------------
        pt_sb = consts.tile([FE, no], f32)
        nc.vector.tensor_copy(pt_sb[:], pt_ps[:])
        with tc.tile_pool(name="fin_sb", bufs=4) as fin_sb:
            for ic in range(nic):
                ptp = scr_ps.tile([P, FE], f32, tag="scr")
                nc.tensor.transpose(ptp[:], pt_sb[:, ic * P:(ic + 1) * P],
                                    identity[0:FE, 0:FE])
                rec = fin_sb.tile([P, 1], f32, tag="rec")
                nc.vector.reciprocal(rec[:], ptp[:, F:FE])
                hp = fin_sb.tile([P, F], f32, tag="hp")
                nc.vector.tensor_scalar(hp[:], ptp[:, 0:F], rec[:], None,
                                        AluOpType.mult)
                # elu(x) = max(x,0) + exp(min(x,0)) - 1
                mn = fin_sb.tile([P, F], f32, tag="mn")
                nc.vector.tensor_scalar(mn[:], hp[:], 0.0, None, AluOpType.min)
                nc.scalar.activation(mn[:], mn[:], AF.Exp)
                nc.vector.tensor_scalar(hp[:], hp[:], 0.0, None, AluOpType.max)
                ob = fin_sb.tile([P, F], f32, tag="ob")
                nc.vector.scalar_tensor_tensor(
                    ob[:], mn[:], 1.0, hp[:],
                    AluOpType.subtract, AluOpType.add)
                nc.sync.dma_start(out_d[ic * P:(ic + 1) * P, :], ob[:])

    nc.compile()
    return nc


_CACHE = {}


def _get_program(nt, no, **kw):
    key = (nt, no, tuple(sorted(kw.items())))
    if key not in _CACHE:
        _CACHE[key] = build_program(nt, no, **kw)
    return _CACHE[key]


def prepare(inputs, **kw):
    """Build (program, per-core input maps) from full unsharded inputs."""
    import ml_dtypes

    bf16 = ml_dtypes.bfloat16
    input = np.ascontiguousarray(inputs["input"], dtype=np.float32)
    adj = inputs["adj"]
    W = np.ascontiguousarray(inputs["W"], dtype=np.float32)
    a = np.ascontiguousarray(inputs["a"], dtype=np.float32)

    nt = input.shape[0]
    no = nt // N_CORES
    nc = _get_program(nt, no, **kw)

    inputT = input.T.astype(bf16, order="C")
    in_maps = []
    for c in range(N_CORES):
        blk = adj[c * no:(c + 1) * no].T  # [nt, no] view
        m = np.where(blk != 0, np.float32(0.0),
                     np.float32(MASK_NEG)).astype(bf16, order="C")
        in_maps.append({
            "inputT": inputT,
            "inputT_own": np.ascontiguousarray(
                inputT[:, c * no:(c + 1) * no]),
            "maskT": m,
            "W": W,
            "a": a,
        })
    return nc, in_maps


def kernel(input, adj, W, a):
    from concourse.bass_utils import run_bass_kernel_spmd

    nc, in_maps = prepare({"input": input, "adj": adj, "W": W, "a": a})
    res = run_bass_kernel_spmd(nc, in_maps, list(range(N_CORES)))
    return np.concatenate([r["out"] for r in res.results], axis=0)


# revision 6
# speedup vs baseline: 1.3315x; 1.0318x over previous
"""Attention graph convolution (GAT layer) on 8 TRN2 NeuronCores — v3.

Reference computation (all fp32):
    h   = input @ W                      # (N, 64)
    e   = leakyrelu(h@a1 + (h@a2).T)     # (N, N)
    att = softmax(where(adj>0, e, -inf)) # row softmax
    out = elu(att @ h)                   # (N, 64)

Sharding: rows of e/att (= output rows) are split across 8 cores,
no = 1536 rows each.  h (N x 64) is computed on every core (tiny).

Design (v1 476 us -> v2 300 -> v2.1 251 -> this):
  - the adjacency mask is host-baked as an ADDITIVE pre-activation
    offset M[j,i] = 0 (edge) / -150 (no edge), transposed to [j, i] and
    cast bf16.  Masking before the leakyrelu is exact enough:
    exp(lrelu(x-150)) <= e^-24, which is < 1e-10 of any row's softmax
    denominator.  This removes the post-exp mask multiply entirely.
  - a runtime-registered custom DVE op (dve_lrelu_op) fuses the whole
    pre-activation for a chunk into ONE Vector instruction:
        e = max(x, 0.2x),  x = Wh1_i + Wh2_j + M[j,i]
    (replaces tensor_scalar+tensor_scalar+tensor_tensor+mask multiply).
  - a lrelu_act_frac fraction of chunks instead run: tensor_tensor add
    (x+M) then ACT Prelu with the Wh2 bias folded in — balancing DVE
    vs ACT, whose irreducible job is the exp.
  - input.T is host-prepared bf16 and SBUF-resident; each h chunk is a
    single PE matmul; h/Wh2 copies are batched x4 through one PSUM
    tile with 3D-AP copies.
  - everything on-chip is bf16 (fp32 PSUM accumulation): the
    accumulation matmul streams at 1 cycle/row (fp32 is 4).
  - exp runs in x2-chunk sub-batches so the PE's accumulation matmuls
    arrive every ~2 us and HAM keeps the PE at 2.4 GHz.
  - no max-subtraction softmax: |e| < ~30 so U = exp(lrelu(e+M))
    cannot overflow; P.T = h_ext.T @ U with h_ext = [h | 1];
    out = elu(P[:, :64] / P[:, 64]).
"""

import numpy as np

N_TOTAL = 12288
K_IN = 128
F_OUT = 64
N_CORES = 8
ALPHA = 0.2
MASK_NEG = -150.0


def build_program(
    nt: int,            # total nodes (j dim)
    no: int,            # nodes owned by this core (i dim)
    batch: int = 4,     # j-chunks per adjacency DMA / phase1b group
    exp_sub: int = 2,   # j-chunks per exp instruction
    lrelu_act_frac: float = 0.27,  # j-chunk fraction with leakyrelu on ACT
    adjt_bufs: int = 3,
    e_bufs: int = 4,
):
    from contextlib import ExitStack

    import concourse.bass as bass
    import concourse.mybir as mybir
    import concourse.tile as tile
    from concourse import bacc
    from concourse.alu_op_type import AluOpType
    from concourse.masks import make_identity

    import dve_lrelu_op

    LRELU_OP = dve_lrelu_op.get_op()

    f32 = mybir.dt.float32
    bf16 = mybir.dt.bfloat16
    AF = mybir.ActivationFunctionType

    P = 128
    F = F_OUT
    FE = F + 1                    # h columns + ones column
    K = K_IN
    assert nt % P == 0 and no % P == 0
    ncj = nt // P                 # j chunks (128 rows each)
    nic = no // P                 # i chunks (own rows)
    S = 512                       # i split for matmul free dim / psum banks
    ns = no // S
    assert no % S == 0
    B = batch
    NB = ncj // B
    assert ncj % B == 0 and B % exp_sub == 0
    n_act = int(round(lrelu_act_frac * ncj))
    NW = 4                        # inputT DMA split (column windows)

    nc = bacc.Bacc("TRN2", target_bir_lowering=False, debug=False,
                   num_devices=1)

    inpT = nc.dram_tensor("inputT", [K, nt], bf16, kind="ExternalInput")
    inpT_own = nc.dram_tensor("inputT_own", [K, no], bf16,
                              kind="ExternalInput")
    # additive mask, transposed: maskT[j, i] = 0 if adj[i, j] else -150
    maskT = nc.dram_tensor("maskT", [nt, no], bf16, kind="ExternalInput")
    w_d = nc.dram_tensor("W", [K, F], f32, kind="ExternalInput")
    a_d = nc.dram_tensor("a", [2 * F, 1], f32, kind="ExternalInput")
    out_d = nc.dram_tensor("out", [no, F], f32, kind="ExternalOutput")

    with tile.TileContext(nc) as tc, ExitStack() as ctx:
        consts = ctx.enter_context(tc.tile_pool(name="consts", bufs=1))

        identity = consts.tile([P, P], f32)
        make_identity(nc, identity)

        scr_ps = ctx.enter_context(
            tc.tile_pool(name="scr_ps", bufs=2, space="PSUM"))
        p1b_ps = ctx.enter_context(
            tc.tile_pool(name="p1b_ps", bufs=2, space="PSUM"))

        # ---- phase 0: Wa1 = W @ a1, Wa2 = W @ a2 (f32), cast bf16 ----
        wwa2_f = consts.tile([K, FE], f32)     # [W | Wa2]
        nc.sync.dma_start(wwa2_f[:, 0:F], w_d.ap())
        a_row = consts.tile([1, 2 * F], f32)   # a as a single-partition row
        nc.sync.dma_start(a_row[:], a_d.ap().rearrange("n o -> o n"))
        ito_own = consts.tile([K, no], bf16)   # inputT own window
        nc.sync.dma_start(ito_own[:], inpT_own.ap())

        ones_sb = consts.tile([P, P], f32)
        nc.vector.memset(ones_sb[:], 1.0)
        # replicate a across partitions via a K=1 matmul with a ones row
        a_rep = consts.tile([P, 2 * F], f32)
        a_rep_ps = scr_ps.tile([P, 2 * F], f32, tag="scr")
        nc.tensor.matmul(a_rep_ps[:], ones_sb[0:1, :], a_row[:],
                         start=True, stop=True)
        nc.vector.tensor_copy(a_rep[:], a_rep_ps[:])

        wa12_sb = consts.tile([K, 2], f32)
        wtmp = consts.tile([K, F], f32)
        nc.vector.tensor_tensor(wtmp[:], wwa2_f[:, 0:F], a_rep[:, 0:F],
                                AluOpType.mult)
        nc.vector.tensor_reduce(wa12_sb[:, 0:1], wtmp[:],
                                mybir.AxisListType.X, AluOpType.add)
        nc.vector.tensor_tensor(wtmp[:], wwa2_f[:, 0:F], a_rep[:, F:2 * F],
                                AluOpType.mult)
        nc.vector.tensor_reduce(wa12_sb[:, 1:2], wtmp[:],
                                mybir.AxisListType.X, AluOpType.add)
        nc.vector.tensor_copy(wwa2_f[:, F:FE], wa12_sb[:, 1:2])
        wwa2_bf = consts.tile([K, FE], bf16)   # [W | Wa2] bf16
        nc.vector.tensor_copy(wwa2_bf[:], wwa2_f[:])
        # Wa1 replicated to 128 cols, bf16
        wa1_rep_f = consts.tile([K, P], f32)
        nc.vector.tensor_scalar(wa1_rep_f[:], ones_sb[:], wa12_sb[:, 0:1],
                                None, AluOpType.mult)
        wa1_rep = consts.tile([K, P], bf16)
        nc.vector.tensor_copy(wa1_rep[:], wa1_rep_f[:])

        # ---- wh1_rep[p, i] = Wh1[own i] for all p ------------------------
        wh1_rep = consts.tile([P, no], bf16)
        for s in range(ns):
            w1p = scr_ps.tile([P, S], f32, tag="scr")
            nc.tensor.matmul(w1p[:], wa1_rep[:], ito_own[:, s * S:(s + 1) * S],
                             start=True, stop=True)
            nc.vector.tensor_copy(wh1_rep[:, s * S:(s + 1) * S], w1p[:])

        # ---- inputT resident in SBUF (windowed DMA) ----------------------
        ito_sb = consts.tile([K, nt], bf16)
        WCOL = nt // NW
        for w in range(NW):
            nc.sync.dma_start(ito_sb[:, w * WCOL:(w + 1) * WCOL],
                              inpT[:, w * WCOL:(w + 1) * WCOL])

        # ---- phase 1b: h_ext[:, jc, :] = [h | 1], wh2 --------------------
        h_ext = consts.tile([P, ncj, FE], bf16)
        wh2_sb = consts.tile([P, ncj], f32)
        nc.vector.memset(h_ext[:, :, F], 1.0)

        def phase1b_group(b):
            # B chunks' h/Wh2 through one PSUM tile, two 3D-AP copies
            hw_ps = p1b_ps.tile([P, B, FE], f32, tag="p1b")
            for q in range(B):
                jc = b * B + q
                nc.tensor.matmul(hw_ps[:, q, :], ito_sb[:, jc * P:(jc + 1) * P],
                                 wwa2_bf[:], start=True, stop=True)
            nc.vector.tensor_copy(h_ext[:, b * B:(b + 1) * B, 0:F],
                                  hw_ps[:, :, 0:F])
            nc.vector.tensor_copy(wh2_sb[:, b * B:(b + 1) * B],
                                  hw_ps[:, :, F])

        def act_path(jc):
            return (jc * 7919) % ncj < n_act

        # ---- phase 2: main loop over j batches ---------------------------
        pt_pool = ctx.enter_context(
            tc.tile_pool(name="pt_acc", bufs=1, space="PSUM"))
        pt_ps = pt_pool.tile([FE, no], f32)

        with (
            tc.tile_pool(name="adjt", bufs=adjt_bufs) as adjt_pool,
            tc.tile_pool(name="epool", bufs=e_bufs) as e_pool,
        ):
            for b in range(NB):
                adjt = adjt_pool.tile([P, B, no], bf16, tag="adjt")
                nc.sync.dma_start(
                    adjt[:],
                    maskT[b * B * P:(b + 1) * B * P, :].rearrange(
                        "(q p) i -> p q i", p=P))
                phase1b_group(b)
                e_sb = e_pool.tile([P, B, no], bf16, tag="e")
                for qs in range(B // exp_sub):
                    for q in range(qs * exp_sub, (qs + 1) * exp_sub):
                        jc = b * B + q
                        if act_path(jc):
                            # x+M on DVE, then lrelu with Wh2 bias on ACT
                            nc.vector.tensor_tensor(
                                e_sb[:, q, :], wh1_rep[:], adjt[:, q, :],
                                AluOpType.add)
                            nc.scalar.activation(
                                e_sb[:, q, :], e_sb[:, q, :], AF.Prelu,
                                bias=wh2_sb[:, jc:jc + 1],
                                scale=1.0, alpha=ALPHA)
                        else:
                            # one fused DVE op: max(x, 0.2x),
                            # x = Wh1 + Wh2 + M
                            nc.vector._custom_dve(
                                LRELU_OP, out=e_sb[:, q, :], in0=wh1_rep[:],
                                in1=adjt[:, q, :],
                                s0=wh2_sb[:, jc:jc + 1], s1=ALPHA)
                    nc.scalar.activation(
                        e_sb[:, qs * exp_sub:(qs + 1) * exp_sub, :],
                        e_sb[:, qs * exp_sub:(qs + 1) * exp_sub, :], AF.Exp)
                    for q in range(qs * exp_sub, (qs + 1) * exp_sub):
                        jc = b * B + q
                        for s in range(ns):
                            nc.tensor.matmul(pt_ps[:, s * S:(s + 1) * S],
                                             h_ext[:, jc, :],
                                             e_sb[:, q, s * S:(s + 1) * S],
                                             start=(jc == 0),
                                             stop=(jc == ncj - 1))

        # ---- phase 3: out = elu(P[:, :64] / P[:, 64]) --------------------
        pt_sb = consts.tile([FE, no], f32)
        nc.vector.tensor_copy(pt_sb[:], pt_ps[:])
        with tc.tile_pool(name="fin_sb", bufs=4) as fin_sb:
            for ic in range(nic):
                ptp = scr_ps.tile([P, FE], f32, tag="scr")
                nc.tensor.transpose(ptp[:], pt_sb[:, ic * P:(ic + 1) * P],
                                    identity[0:FE, 0:FE])
                rec = fin_sb.tile([P, 1], f32, tag="rec")
                nc.vector.reciprocal(rec[:], ptp[:, F:FE])
                hp = fin_sb.tile([P, F], f32, tag="hp")
                nc.vector.tensor_scalar(hp[:], ptp[:, 0:F], rec[:], None,
                                        AluOpType.mult)
                # elu(x) = max(x,0) + exp(min(x,0)) - 1
                mn = fin_sb.tile([P, F], f32, tag="mn")
                nc.vector.tensor_scalar(mn[:], hp[:], 0.0, None, AluOpType.min)
                nc.scalar.activation(mn[:], mn[:], AF.Exp)
                nc.vector.tensor_scalar(hp[:], hp[:], 0.0, None, AluOpType.max)
                ob = fin_sb.tile([P, F], f32, tag="ob")
                nc.vector.scalar_tensor_tensor(
                    ob[:], mn[:], 1.0, hp[:],
                    AluOpType.subtract, AluOpType.add)
                nc.sync.dma_start(out_d[ic * P:(ic + 1) * P, :], ob[:])

    nc.compile()
    return nc


_CACHE = {}


def _get_program(nt, no, **kw):
    key = (nt, no, tuple(sorted(kw.items())))
    if key not in _CACHE:
        _CACHE[key] = build_program(nt, no, **kw)
    return _CACHE[key]


def prepare(inputs, **kw):
    """Build (program, per-core input maps) from full unsharded inputs."""
    import ml_dtypes

    bf16 = ml_dtypes.bfloat16
    input = np.ascontiguousarray(inputs["input"], dtype=np.float32)
    adj = inputs["adj"]
    W = np.ascontiguousarray(inputs["W"], dtype=np.float32)
    a = np.ascontiguousarray(inputs["a"], dtype=np.float32)

    nt = input.shape[0]
    no = nt // N_CORES
    nc = _get_program(nt, no, **kw)

    inputT = input.T.astype(bf16, order="C")
    in_maps = []
    for c in range(N_CORES):
        blk = adj[c * no:(c + 1) * no].T  # [nt, no] view
        m = np.where(blk != 0, np.float32(0.0),
                     np.float32(MASK_NEG)).astype(bf16, order="C")
        in_maps.append({
            "inputT": inputT,
            "inputT_own": np.ascontiguousarray(
                inputT[:, c * no:(c + 1) * no]),
            "maskT": m,
            "W": W,
            "a": a,
        })
    return nc, in_maps


def kernel(input, adj, W, a):
    from concourse.bass_utils import run_bass_kernel_spmd

    nc, in_maps = prepare({"input": input, "adj": adj, "W": W, "a": a})
    res = run_bass_kernel_spmd(nc, in_maps, list(range(N_CORES)))
    return np.concatenate([r["out"] for r in res.results], axis=0)


# revision 9
# speedup vs baseline: 1.3493x; 1.0133x over previous
"""Attention graph convolution (GAT layer) on 8 TRN2 NeuronCores — v3.

Reference computation (all fp32):
    h   = input @ W                      # (N, 64)
    e   = leakyrelu(h@a1 + (h@a2).T)     # (N, N)
    att = softmax(where(adj>0, e, -inf)) # row softmax
    out = elu(att @ h)                   # (N, 64)

Sharding: rows of e/att (= output rows) are split across 8 cores,
no = 1536 rows each.  h (N x 64) is computed on every core (tiny).

Design (v1 476 us -> v2 300 -> v2.1 251 -> this):
  - the adjacency mask is host-baked as an ADDITIVE pre-activation
    offset M[j,i] = 0 (edge) / -150 (no edge), transposed to [j, i] and
    cast bf16.  Masking before the leakyrelu is exact enough:
    exp(lrelu(x-150)) <= e^-24, which is < 1e-10 of any row's softmax
    denominator.  This removes the post-exp mask multiply entirely.
  - a runtime-registered custom DVE op (dve_lrelu_op) fuses the whole
    pre-activation for a chunk into ONE Vector instruction:
        e = max(x, 0.2x),  x = Wh1_i + Wh2_j + M[j,i]
    (replaces tensor_scalar+tensor_scalar+tensor_tensor+mask multiply).
  - a lrelu_act_frac fraction of chunks instead run: tensor_tensor add
    (x+M) then ACT Prelu with the Wh2 bias folded in — balancing DVE
    vs ACT, whose irreducible job is the exp.
  - input.T is host-prepared bf16 and SBUF-resident; each h chunk is a
    single PE matmul; h/Wh2 copies are batched x4 through one PSUM
    tile with 3D-AP copies.
  - everything on-chip is bf16 (fp32 PSUM accumulation): the
    accumulation matmul streams at 1 cycle/row (fp32 is 4).
  - exp runs in x2-chunk sub-batches so the PE's accumulation matmuls
    arrive every ~2 us and HAM keeps the PE at 2.4 GHz.
  - no max-subtraction softmax: |e| < ~30 so U = exp(lrelu(e+M))
    cannot overflow; P.T = h_ext.T @ U with h_ext = [h | 1];
    out = elu(P[:, :64] / P[:, 64]).
"""

import numpy as np

N_TOTAL = 12288
K_IN = 128
F_OUT = 64
N_CORES = 8
ALPHA = 0.2
MASK_NEG = -150.0


def build_program(
    nt: int,            # total nodes (j dim)
    no: int,            # nodes owned by this core (i dim)
    batch: int = 4,     # j-chunks per adjacency DMA / phase1b group
    exp_sub: int = 2,   # j-chunks per exp instruction
    lrelu_act_frac: float = 0.27,  # j-chunk fraction with leakyrelu on ACT
    adjt_bufs: int = 3,
    e_bufs: int = 4,
):
    from contextlib import ExitStack

    import concourse.bass as bass
    import concourse.mybir as mybir
    import concourse.tile as tile
    from concourse import bacc
    from concourse.alu_op_type import AluOpType
    from concourse.masks import make_identity

    import dve_lrelu_op

    LRELU_OP = dve_lrelu_op.get_op()

    f32 = mybir.dt.float32
    bf16 = mybir.dt.bfloat16
    AF = mybir.ActivationFunctionType

    P = 128
    F = F_OUT
    FE = F + 1                    # h columns + ones column
    K = K_IN
    assert nt % P == 0 and no % P == 0
    ncj = nt // P                 # j chunks (128 rows each)
    nic = no // P                 # i chunks (own rows)
    S = 512                       # i split for matmul free dim / psum banks
    ns = no // S
    assert no % S == 0
    B = batch
    NB = ncj // B
    assert ncj % B == 0 and B % exp_sub == 0
    n_act = int(round(lrelu_act_frac * ncj))
    NW = 4                        # inputT DMA split (column windows)

    nc = bacc.Bacc("TRN2", target_bir_lowering=False, debug=False,
                   num_devices=1)

    inpT = nc.dram_tensor("inputT", [K, nt], bf16, kind="ExternalInput")
    inpT_own = nc.dram_tensor("inputT_own", [K, no], bf16,
                              kind="ExternalInput")
    # additive mask, transposed: maskT[j, i] = 0 if adj[i, j] else -150
    maskT = nc.dram_tensor("maskT", [nt, no], bf16, kind="ExternalInput")
    w_d = nc.dram_tensor("W", [K, F], f32, kind="ExternalInput")
    a_d = nc.dram_tensor("a", [2 * F, 1], f32, kind="ExternalInput")
    out_d = nc.dram_tensor("out", [no, F], f32, kind="ExternalOutput")

    with tile.TileContext(nc) as tc, ExitStack() as ctx:
        consts = ctx.enter_context(tc.tile_pool(name="consts", bufs=1))

        identity = consts.tile([P, P], f32)
        make_identity(nc, identity)

        scr_ps = ctx.enter_context(
            tc.tile_pool(name="scr_ps", bufs=2, space="PSUM"))
        p1b_ps = ctx.enter_context(
            tc.tile_pool(name="p1b_ps", bufs=2, space="PSUM"))

        # ---- phase 0: Wa1 = W @ a1, Wa2 = W @ a2 (f32), cast bf16 ----
        wwa2_f = consts.tile([K, FE], f32)     # [W | Wa2]
        nc.sync.dma_start(wwa2_f[:, 0:F], w_d.ap())
        a_row = consts.tile([1, 2 * F], f32)   # a as a single-partition row
        nc.sync.dma_start(a_row[:], a_d.ap().rearrange("n o -> o n"))
        ito_own = consts.tile([K, no], bf16)   # inputT own window
        nc.sync.dma_start(ito_own[:], inpT_own.ap())

        ones_sb = consts.tile([P, P], f32)
        nc.vector.memset(ones_sb[:], 1.0)
        # replicate a across partitions via a K=1 matmul with a ones row
        a_rep = consts.tile([P, 2 * F], f32)
        a_rep_ps = scr_ps.tile([P, 2 * F], f32, tag="scr")
        nc.tensor.matmul(a_rep_ps[:], ones_sb[0:1, :], a_row[:],
                         start=True, stop=True)
        nc.vector.tensor_copy(a_rep[:], a_rep_ps[:])

        wa12_sb = consts.tile([K, 2], f32)
        wtmp = consts.tile([K, F], f32)
        nc.vector.tensor_tensor(wtmp[:], wwa2_f[:, 0:F], a_rep[:, 0:F],
                                AluOpType.mult)
        nc.vector.tensor_reduce(wa12_sb[:, 0:1], wtmp[:],
                                mybir.AxisListType.X, AluOpType.add)
        nc.vector.tensor_tensor(wtmp[:], wwa2_f[:, 0:F], a_rep[:, F:2 * F],
                                AluOpType.mult)
        nc.vector.tensor_reduce(wa12_sb[:, 1:2], wtmp[:],
                                mybir.AxisListType.X, AluOpType.add)
        nc.vector.tensor_copy(wwa2_f[:, F:FE], wa12_sb[:, 1:2])
        wwa2_bf = consts.tile([K, FE], bf16)   # [W | Wa2] bf16
        nc.vector.tensor_copy(wwa2_bf[:], wwa2_f[:])
        # Wa1 replicated to 128 cols, bf16
        wa1_rep_f = consts.tile([K, P], f32)
        nc.vector.tensor_scalar(wa1_rep_f[:], ones_sb[:], wa12_sb[:, 0:1],
                                None, AluOpType.mult)
        wa1_rep = consts.tile([K, P], bf16)
        nc.vector.tensor_copy(wa1_rep[:], wa1_rep_f[:])

        # ---- wh1_rep[p, i] = Wh1[own i] for all p ------------------------
        wh1_rep = consts.tile([P, no], bf16)
        for s in range(ns):
            w1p = scr_ps.tile([P, S], f32, tag="scr")
            nc.tensor.matmul(w1p[:], wa1_rep[:], ito_own[:, s * S:(s + 1) * S],
                             start=True, stop=True)
            nc.vector.tensor_copy(wh1_rep[:, s * S:(s + 1) * S], w1p[:])

        # ---- adjacency prefetch + exp-table warmup -----------------------
        adjt_pool = ctx.enter_context(tc.tile_pool(name="adjt",
                                                   bufs=adjt_bufs))
        adjt_tiles = {}

        def adjt_fetch(b):
            t = adjt_pool.tile([P, B, no], bf16, tag="adjt")
            nc.sync.dma_start(
                t[:],
                maskT[b * B * P:(b + 1) * B * P, :].rearrange(
                    "(q p) i -> p q i", p=P))
            adjt_tiles[b] = t

        for b in range(min(adjt_bufs - 1, ncj // B)):
            adjt_fetch(b)
        # load the exp table set during the prologue, not at first real exp
        nc.scalar.activation(wtmp[:, 0:1], wa12_sb[:, 0:1], AF.Exp)

        # ---- inputT resident in SBUF (windowed DMA) ----------------------
        ito_sb = consts.tile([K, nt], bf16)
        WCOL = nt // NW
        for w in range(NW):
            nc.sync.dma_start(ito_sb[:, w * WCOL:(w + 1) * WCOL],
                              inpT[:, w * WCOL:(w + 1) * WCOL])

        # ---- phase 1b: h_ext[:, jc, :] = [h | 1], wh2 --------------------
        h_ext = consts.tile([P, ncj, FE], bf16)
        wh2_sb = consts.tile([P, ncj], f32)
        nc.vector.memset(h_ext[:, :, F], 1.0)

        def phase1b_group(b):
            # B chunks' h/Wh2 through one PSUM tile, two 3D-AP copies
            hw_ps = p1b_ps.tile([P, B, FE], f32, tag="p1b")
            for q in range(B):
                jc = b * B + q
                nc.tensor.matmul(hw_ps[:, q, :], ito_sb[:, jc * P:(jc + 1) * P],
                                 wwa2_bf[:], start=True, stop=True)
            nc.vector.tensor_copy(h_ext[:, b * B:(b + 1) * B, 0:F],
                                  hw_ps[:, :, 0:F])
            nc.vector.tensor_copy(wh2_sb[:, b * B:(b + 1) * B],
                                  hw_ps[:, :, F])

        def act_path(jc):
            return (jc * 7919) % ncj < n_act

        # ---- phase 2: main loop over j batches ---------------------------
        pt_pool = ctx.enter_context(
            tc.tile_pool(name="pt_acc", bufs=1, space="PSUM"))
        pt_ps = pt_pool.tile([FE, no], f32)

        with (
            tc.tile_pool(name="epool", bufs=e_bufs) as e_pool,
        ):
            for b in range(NB):
                if b not in adjt_tiles:
                    adjt_fetch(b)
                adjt = adjt_tiles.pop(b)
                nf = b + adjt_bufs - 1
                if nf < NB and nf not in adjt_tiles:
                    adjt_fetch(nf)
                phase1b_group(b)
                e_sb = e_pool.tile([P, B, no], bf16, tag="e")
                for qs in range(B // exp_sub):
                    for q in range(qs * exp_sub, (qs + 1) * exp_sub):
                        jc = b * B + q
                        if act_path(jc):
                            # x+M on DVE, then lrelu with Wh2 bias on ACT
                            nc.vector.tensor_tensor(
                                e_sb[:, q, :], wh1_rep[:], adjt[:, q, :],
                                AluOpType.add)
                            nc.scalar.activation(
                                e_sb[:, q, :], e_sb[:, q, :], AF.Prelu,
                                bias=wh2_sb[:, jc:jc + 1],
                                scale=1.0, alpha=ALPHA)
                        else:
                            # one fused DVE op: max(x, 0.2x),
                            # x = Wh1 + Wh2 + M
                            nc.vector._custom_dve(
                                LRELU_OP, out=e_sb[:, q, :], in0=wh1_rep[:],
                                in1=adjt[:, q, :],
                                s0=wh2_sb[:, jc:jc + 1], s1=ALPHA)
                    nc.scalar.activation(
                        e_sb[:, qs * exp_sub:(qs + 1) * exp_sub, :],
                        e_sb[:, qs * exp_sub:(qs + 1) * exp_sub, :], AF.Exp)
                    for q in range(qs * exp_sub, (qs + 1) * exp_sub):
                        jc = b * B + q
                        for s in range(ns):
                            nc.tensor.matmul(pt_ps[:, s * S:(s + 1) * S],
                                             h_ext[:, jc, :],
                                             e_sb[:, q, s * S:(s + 1) * S],
                                             start=(jc == 0),
                                             stop=(jc == ncj - 1))

        # ---- phase 3: out = elu(P[:, :64] / P[:, 64]) --------------------
        pt_sb = consts.tile([FE, no], f32)
        with tc.tile_pool(name="fin_sb", bufs=4) as fin_sb:
            for ic in range(nic):
                nc.vector.tensor_copy(pt_sb[:, ic * P:(ic + 1) * P],
                                      pt_ps[:, ic * P:(ic + 1) * P])
                ptp = scr_ps.tile([P, FE], f32, tag="scr")
                nc.tensor.transpose(ptp[:], pt_sb[:, ic * P:(ic + 1) * P],
                                    identity[0:FE, 0:FE])
                rec = fin_sb.tile([P, 1], f32, tag="rec")
                nc.vector.reciprocal(rec[:], ptp[:, F:FE])
                hp = fin_sb.tile([P, F], f32, tag="hp")
                nc.vector.tensor_scalar(hp[:], ptp[:, 0:F], rec[:], None,
                                        AluOpType.mult)
                # elu(x) = max(x,0) + exp(min(x,0)) - 1
                mn = fin_sb.tile([P, F], f32, tag="mn")
                nc.vector.tensor_scalar(mn[:], hp[:], 0.0, None, AluOpType.min)
                nc.scalar.activation(mn[:], mn[:], AF.Exp)
                nc.vector.tensor_scalar(hp[:], hp[:], 0.0, None, AluOpType.max)
                ob = fin_sb.tile([P, F], f32, tag="ob")
                nc.vector.scalar_tensor_tensor(
                    ob[:], mn[:], 1.0, hp[:],
                    AluOpType.subtract, AluOpType.add)
                nc.sync.dma_start(out_d[ic * P:(ic + 1) * P, :], ob[:])

    nc.compile()
    return nc


_CACHE = {}


def _get_program(nt, no, **kw):
    key = (nt, no, tuple(sorted(kw.items())))
    if key not in _CACHE:
        _CACHE[key] = build_program(nt, no, **kw)
    return _CACHE[key]


def prepare(inputs, **kw):
    """Build (program, per-core input maps) from full unsharded inputs."""
    import ml_dtypes

    bf16 = ml_dtypes.bfloat16
    input = np.ascontiguousarray(inputs["input"], dtype=np.float32)
    adj = inputs["adj"]
    W = np.ascontiguousarray(inputs["W"], dtype=np.float32)
    a = np.ascontiguousarray(inputs["a"], dtype=np.float32)

    nt = input.shape[0]
    no = nt // N_CORES
    nc = _get_program(nt, no, **kw)

    inputT = input.T.astype(bf16, order="C")
    in_maps = []
    for c in range(N_CORES):
        blk = adj[c * no:(c + 1) * no].T  # [nt, no] view
        m = np.where(blk != 0, np.float32(0.0),
                     np.float32(MASK_NEG)).astype(bf16, order="C")
        in_maps.append({
            "inputT": inputT,
            "inputT_own": np.ascontiguousarray(
                inputT[:, c * no:(c + 1) * no]),
            "maskT": m,
            "W": W,
            "a": a,
        })
    return nc, in_maps


def kernel(input, adj, W, a):
    from concourse.bass_utils import run_bass_kernel_spmd

    nc, in_maps = prepare({"input": input, "adj": adj, "W": W, "a": a})
    res = run_bass_kernel_spmd(nc, in_maps, list(range(N_CORES)))
    return np.concatenate([r["out"] for r in res.results], axis=0)
